# revision 7
# baseline (speedup 1.0000x reference)
"""LinearSelfAttention kernel for TRN2 (8 NeuronCores, batch-parallel).

Key identity: with Hn = H[:, :n] (mask drops column n from the s-sum),
    attn = P H mask(H^T Q H) = C H,   C = P G Q,   G = Hn Hn^T  (257x257)
so  out = H + C H / n = H + Ct^T H / n,  Ct = C^T = Q^T G P^T.

Device computes ONLY the attention term A = Ct^T H (256-dim blocks); the
host adds H exactly in f32 plus thin rank-1 edge corrections (the e=256 /
a=256 slices of the chain, output row d=256, column t=2048).  This removes
every K=1 edge matmul AND the bf16 H input copy:

 - G via fp8e4m3 DoubleRow (K=256/pass), host-transposed Hn (fp8, 1.1MB).
 - Small chain V = G P^T, Ct = Q^T V in bf16 (K=128 passes, 256-blocks).
 - Ct evicted PSUM->SBUF as fp8 in DoubleRow weight layout; S4 A = Ct^T H
   runs fp8 DoubleRow with Ct stationary: 8 passes of N=512 per sample
   (vs 16 bf16 + 8 edge passes in the old scheme).  H arrives as a second
   fp8 copy in natural layout (1.05MB vs 2.1MB bf16).  Precision is fine
   because Ct ~ O(1) entries scale the fp8 H error by ~Ct*dH ~ 1e-4 of
   the output scale.
 - Input DMA: Ht split sync/scalar so G(0) starts ~3.5us in; qpi on
   gpsimd.  Y stores round-robin sync/gpsimd, fine-grained near the end
   so the post-compute drain is short.

Sharding: data-parallel over batch, 2 samples per core.
"""

import sys

sys.path.insert(0, "/opt/trn_rl_repo")

import numpy as np
import ml_dtypes

B, D1, N1 = 16, 257, 2049  # batch, d+1, n+1
N = N1 - 1  # 2048
NCORES = 8
BPC = B // NCORES  # samples per core

NT8 = N // 256  # 8 double-row s-tiles of the transposed Hn
DPAD = 272  # fp8 DR LDWEIGHTS: step between the 2 K-subtiles must be %16==0
HPAD = 2064  # padded t-dim of the natural-layout fp8 H (%16==0)
TCH = [(i * 512, 512) for i in range(4)]  # t=2048 column done on host
NWARM = 24

_cached = {}


def _build():
    import concourse.bass as bass
    import concourse.tile as tile
    from concourse import bacc, mybir
    from contextlib import ExitStack

    f32 = mybir.dt.float32
    bf16 = mybir.dt.bfloat16
    f8 = mybir.dt.float8e4
    DR = mybir.MatmulPerfMode.DoubleRow

    nc = bacc.Bacc("TRN2", target_bir_lowering=False, debug=False, num_devices=NCORES)

    Ht_d = nc.declare_dram_parameter("Ht", [BPC, NT8, 128, 2, DPAD], f8, isOutput=False)
    Hf_d = nc.declare_dram_parameter("Hf", [BPC, 128, 2, HPAD], f8, isOutput=False)
    QPI_d = nc.declare_dram_parameter("QPI", [256, 2 * D1], bf16, isOutput=False)
    Y_d = nc.declare_dram_parameter("Y", [BPC, 256, N], bf16, isOutput=True)

    with tile.TileContext(nc) as tc:
        with ExitStack() as ctx:
            const = ctx.enter_context(tc.tile_pool(name="const", bufs=1))
            htp = ctx.enter_context(tc.tile_pool(name="htp", bufs=2))
            hfp = ctx.enter_context(tc.tile_pool(name="hfp", bufs=2))
            sq = ctx.enter_context(tc.tile_pool(name="sq", bufs=2))
            yp = ctx.enter_context(tc.tile_pool(name="yp", bufs=2))

            # ---- input DMAs.  Ht (the G operand) first, 3-way split across
            # all queues so sample 0's first s-tiles land ASAP and G(0)
            # streams against arrival; qpi + the fp8 natural-layout H after.
            ht = [None] * BPC
            hf = [None] * BPC
            qpi = []
            for b in range(BPC):
                ht[b] = htp.tile([128, NT8, 2, DPAD], f8, tag="ht", name=f"ht{b}")
            # piece assignment matches engine issue latency to consumption
            # order: sync issues earliest (first s-tiles), scalar issues
            # ~1.3us late (ACT table load) so it gets the last-consumed tiles
            nc.sync.dma_start(ht[0][:, 0:3, :, :], Ht_d[0, 0:3])
            nc.gpsimd.dma_start(ht[0][:, 3:6, :, :], Ht_d[0, 3:6])
            nc.scalar.dma_start(ht[0][:, 6:8, :, :], Ht_d[0, 6:8])
            nc.scalar.dma_start(ht[1][:, 0:3, :, :], Ht_d[1, 0:3])
            nc.gpsimd.dma_start(ht[1][:, 3:6, :, :], Ht_d[1, 3:6])
            nc.sync.dma_start(ht[1][:, 6:8, :, :], Ht_d[1, 6:8])
            for c in range(2):
                t = const.tile([128, 2 * D1], bf16, tag=f"qpi{c}", name=f"qpi{c}")
                nc.gpsimd.dma_start(t[:, :], QPI_d[c * 128 : (c + 1) * 128, :])
                qpi.append(t)
            for b in range(BPC):
                t = hfp.tile([128, 2, HPAD], f8, tag="hf", name=f"hf{b}")
                eng = nc.sync if b == 0 else nc.scalar
                eng.dma_start(t[:, :, :N1], Hf_d[b, :, :, :N1])
                hf[b] = t

            # ---- PE warmup: ride the clock ramp until the first tile lands
            wsb = const.tile([128, 128], bf16, tag="wsb", name="wsb")
            nc.vector.memset(wsb[:, :], 0.0)
            with tc.tile_pool(name="wp", bufs=1, space="PSUM") as wp:
                wps = wp.tile([128, 512], f32, tag="wps", name="warm_ps")
                for i in range(NWARM):
                    nc.tensor.matmul(
                        wps[:, 0:128],
                        wsb[:, :],
                        wsb[:, :],
                        start=(i == 0),
                        stop=(i == NWARM - 1),
                    )

            with (
                tc.tile_pool(name="ppa", bufs=4, space="PSUM") as ppa,
                tc.tile_pool(name="ppb", bufs=4, space="PSUM") as ppb,
            ):
                gsb = [None] * BPC
                vsb = [None] * BPC
                ct8 = [None] * BPC

                def g_stage(b):
                    # ---- G = Hn Hn^T (fp8 DoubleRow, K=256 per pass)
                    gA = ppa.tile([128, 512], f32, tag="A", name=f"gA{b}")
                    gB = ppb.tile([128, 512], f32, tag="B", name=f"gB{b}")
                    regions = [gA[:, 0:D1], gB[:, 0:D1]]
                    for st in range(NT8):
                        for ac in range(2):
                            nc.tensor.matmul(
                                regions[ac][:, :],
                                ht[b][:, st, :, ac * 128 : (ac + 1) * 128],
                                ht[b][:, st, :, :D1],
                                start=(st == 0),
                                stop=(st == NT8 - 1),
                                perf_mode=DR,
                            )
                    gs = []
                    for ac in range(2):
                        t = sq.tile([128, D1], bf16, tag=f"g{ac}", name=f"gs{b}_{ac}")
                        eng = nc.scalar.copy if ac % 2 == 0 else nc.vector.tensor_copy
                        eng(t[:, :], regions[ac][:, :])
                        gs.append(t)
                    gsb[b] = gs

                def v_stage(b):
                    # ---- V = G P^T  (G symmetric: lhsT slices G directly)
                    vA = ppa.tile([128, 512], f32, tag="A", name=f"vA{b}")
                    vB = ppb.tile([128, 512], f32, tag="B", name=f"vB{b}")
                    regions = [vA[:, 0:D1], vB[:, 0:D1]]
                    for kb in range(2):
                        for am in range(2):
                            nc.tensor.matmul(
                                regions[am][:, :],
                                gsb[b][kb][:, am * 128 : (am + 1) * 128],
                                qpi[kb][:, D1 : 2 * D1],
                                start=(kb == 0),
                                stop=(kb == 1),
                            )
                    vs = []
                    for am in range(2):
                        t = sq.tile([128, D1], bf16, tag=f"v{am}", name=f"vs{b}_{am}")
                        eng = nc.scalar.copy if am % 2 == 1 else nc.vector.tensor_copy
                        eng(t[:, :], regions[am][:, :])
                        vs.append(t)
                    vsb[b] = vs

                def ct_stage(b):
                    # ---- Ct = Q^T V  (= C^T = n * Ct; host divides by n)
                    cA = ppa.tile([128, 512], f32, tag="A", name=f"cA{b}")
                    cB = ppb.tile([128, 512], f32, tag="B", name=f"cB{b}")
                    cregions = [cA[:, 0:D1], cB[:, 0:D1]]
                    for ka in range(2):
                        for em in range(2):
                            nc.tensor.matmul(
                                cregions[em][:, :],
                                qpi[ka][:, em * 128 : (em + 1) * 128],
                                vsb[b][ka][:, :],
                                start=(ka == 0),
                                stop=(ka == 1),
                            )
                    # evict as fp8 in the DoubleRow weight layout:
                    # ct8[p, i, d] = Ct[i*128+p, d]
                    t = sq.tile([128, 2, DPAD], f8, tag="ct8", name=f"ct8_{b}")
                    nc.scalar.copy(t[:, 0, 0:256], cregions[0][:, 0:256])
                    nc.vector.tensor_copy(t[:, 1, 0:256], cregions[1][:, 0:256])
                    ct8[b] = t

                def s4_stage(b):
                    # ---- A[d<256] = Ct^T H, fp8 DoubleRow, Ct stationary.
                    # Evictions alternate engines per chunk (they, not the
                    # matmuls, pace this stage); stores are per-chunk eager,
                    # with the final chunk split in half for a short drain.
                    y = [
                        yp.tile([128, N], bf16, tag=f"y{dc}", name=f"y{b}_{dc}")
                        for dc in range(2)
                    ]
                    pools = [ppa, ppb]
                    tags = ["A", "B"]
                    for dc in range(2):
                        for ti, (toff, tsz) in enumerate(TCH):
                            p = pools[dc].tile(
                                [128, 512], f32, tag=tags[dc], name=f"p{b}_{dc}_{ti}"
                            )
                            nc.tensor.matmul(
                                p[:128, :tsz],
                                ct8[b][:, :, dc * 128 : (dc + 1) * 128],
                                hf[b][:, :, toff : toff + tsz],
                                start=True,
                                stop=True,
                                perf_mode=DR,
                            )
                            eng = (
                                nc.scalar.copy
                                if (dc + ti) % 2 == 0
                                else nc.vector.tensor_copy
                            )
                            q = nc.sync if dc == 0 else nc.gpsimd
                            dsl = slice(dc * 128, (dc + 1) * 128)
                            last = b == BPC - 1 and ti == 3
                            if not last:
                                eng(y[dc][:, toff : toff + tsz], p[:128, :tsz])
                                q.dma_start(
                                    Y_d[b, dsl, toff : toff + tsz],
                                    y[dc][:, toff : toff + tsz],
                                )
                            else:
                                # final chunk: halves on both engines, store
                                # each as it lands
                                nc.scalar.copy(y[dc][:, 1536:1792], p[:128, 0:256])
                                nc.vector.tensor_copy(
                                    y[dc][:, 1792:N], p[:128, 256:512]
                                )
                                q.dma_start(Y_d[b, dsl, 1536:1792], y[dc][:, 1536:1792])
                                q.dma_start(Y_d[b, dsl, 1792:N], y[dc][:, 1792:N])

                # emission order: samples fully pipelined so every PE stage's
                # operands were evicted during the previous stage
                g_stage(0)
                g_stage(1)
                v_stage(0)
                v_stage(1)
                ct_stage(0)
                ct_stage(1)
                s4_stage(0)
                s4_stage(1)

    nc.compile()
    return nc


def _prep_in_maps(H, P, Q):
    bf = ml_dtypes.bfloat16
    f8 = ml_dtypes.float8_e4m3
    H = np.ascontiguousarray(H, dtype=np.float32)
    # G operand: [st, p, i, d] with s = st*256 + i*128 + p
    Ht = np.swapaxes(H[:, :, :N], 1, 2).reshape(B, NT8, 2, 128, D1)
    Ht8 = np.zeros((B, NT8, 128, 2, DPAD), dtype=f8)
    Ht8[..., :D1] = np.swapaxes(Ht, 2, 3).astype(f8)
    # S4 operand: natural layout fp8, [p, c, t] = H[c*128+p, t]
    Hf8 = np.zeros((B, 128, 2, HPAD), dtype=f8)
    Hf8[..., :N1] = np.swapaxes(H[:, :256, :].reshape(B, 2, 128, N1), 1, 2).astype(f8)
    QPI = np.ascontiguousarray(
        np.concatenate([Q, P.T], axis=1)[:256].astype(bf)
    )
    return [
        {
            "Ht": Ht8[c * BPC : (c + 1) * BPC],
            "Hf": Hf8[c * BPC : (c + 1) * BPC],
            "QPI": QPI,
        }
        for c in range(NCORES)
    ]


def kernel(H, P, Q):
    from concourse.bass_utils import run_bass_kernel_spmd

    if "nc" not in _cached:
        _cached["nc"] = _build()
    nc = _cached["nc"]

    in_maps = _prep_in_maps(H, P, Q)
    for attempt in range(3):
        res = run_bass_kernel_spmd(nc, in_maps, list(range(NCORES)))
        adev = np.concatenate(
            [res.results[c]["Y"].astype(np.float32) for c in range(NCORES)], axis=0
        )
        if np.isfinite(adev).all():
            break

    H = np.ascontiguousarray(H, dtype=np.float32)
    P = np.ascontiguousarray(P, dtype=np.float32)
    Q = np.ascontiguousarray(Q, dtype=np.float32)
    Hn = H[:, :, :N]
    Hm = Hn[:, :256, :]  # rows 0..255

    out = np.empty((B, D1, N1), dtype=np.float32)
    # device part: A over (m<256, e<256) chain blocks
    out[:, :256, :N] = H[:, :256, :N] + adev / N

    # ---- host edge corrections (all thin O(n d) rank-1 terms, exact f32)
    # g256[e] = G[256, e]; v256[d] = V[256, d]; er256[d] = C[d,256]/n
    g256 = np.einsum("bds,bs->bd", Hn, Hn[:, 256, :])
    v256 = g256 @ P.T
    t1 = np.einsum("bds,d->bs", Hn, Q[:, 256])
    r = np.einsum("bds,bs->bd", Hn, t1)  # = G @ Q[:,256]
    er256 = (r @ P.T) / N
    # m=256 term of A: er256 (x) H[256, :]
    out[:, :256, :N] += er256[:, :256, None] * Hn[:, None, 256, :]
    # a=256 / e=256 chain slices for m<256:
    #   corr[m,d] = P[d,256] * (sum_a<256 Q[a,m] g256[a])/n + Q[256,m] v256[d]/n
    qg = np.einsum("ba,am->bm", g256[:, :256], Q[:256, :256])
    z1 = np.einsum("bm,bmt->bt", qg, Hm) / N
    z2 = np.einsum("m,bmt->bt", Q[256, :256], Hm) / N
    out[:, :256, :N] += P[None, :256, 256, None] * z1[:, None, :]
    out[:, :256, :N] += v256[:, :256, None] * z2[:, None, :]

    # output row d=256 exactly, on host (fp32)
    u = np.einsum("bds,d->bs", Hn, P[256, :])
    v = np.einsum("bds,bs->bd", Hn, u)  # = G @ P[256,:] per sample
    c256 = v @ Q  # = C[256, :] per sample
    out[:, 256, :] = H[:, 256, :] + np.einsum("bd,bdt->bt", c256, H) / N
    # exact column t=2048 on host: C @ hcol = P (G (Q hcol))
    hcol = H[:, :, N]  # (B, 257)
    w1 = hcol @ Q.T  # (Q hcol)[a]
    w2 = np.einsum("bds,bs->bd", Hn, np.einsum("bds,bd->bs", Hn, w1))  # G w1
    w3 = w2 @ P.T
    out[:, :256, N] = hcol[:, :256] + w3[:, :256] / N
    return out
